# revision 17
# baseline (speedup 1.0000x reference)
"""Trainium2 Bass kernel for nn_AttentiveTransformer
(fc -> GhostBN -> *prior -> sparsemax), 8-core data-parallel over batch.

Matmul is fp16 x fp16 (full PE rate, 1 cyc/row; fp32 PSUM accumulate)
with N=512 moving tiles so weight loads amortize.  All intermediates
(H, prior, z) are fp16; stats and output are f32.  GhostBN stats ride
DVE bn_stats off PSUM; scale/shift math runs on Pool with a
magic-constant rsqrt (no ACT table switches); the fused BN-apply +
PSUM-evacuate runs on ACT (Identity only).  Sparsemax: top-8 per
512-quarter compaction (DVE InstMax) + 5 Newton iterations, finals as
one batched ACT Relu block.  Simulated end-to-end rel-l2 ~2.5e-3
(gate 2e-2).

kernel(**inputs) takes FULL f32 inputs, returns FULL f32 output.
"""
import numpy as np
from contextlib import ExitStack

import concourse.bacc as bacc
import concourse.tile as tile
import concourse.mybir as mybir
from concourse.bass_utils import run_bass_kernel_spmd
from concourse import masks

f32 = mybir.dt.float32
fp16 = mybir.dt.float16
i32 = mybir.dt.int32
AF = mybir.ActivationFunctionType
ALU = mybir.AluOpType
AX = mybir.AxisListType

N_CORES = 8
B_FULL = 16384
D = 2048
BL = B_FULL // N_CORES    # 2048 rows per core
P = 128
KT = 16                   # k-tiles of 128 over D_in
MT = 16                   # m-tiles of 128 over D_out
SEG = 512                 # batch rows per segment
NSEG = BL // SEG          # 4
NU = SEG // P             # 4 u-tiles (128 rows) per segment
NIT = 5
EPS = 1e-5
MAGIC = 0x5F3759DF
RSQRT_MAGIC = True


def _body(nc, tc, ctx, X16, W16, PRI, Gd, Bd, OUT, repeat=1, mode='full'):
    sb_const = ctx.enter_context(tc.tile_pool(name="const", bufs=1))
    wt_pool = ctx.enter_context(tc.tile_pool(name="wt", bufs=1))
    xt_pool = ctx.enter_context(tc.tile_pool(name="xt", bufs=2))
    h_pool = ctx.enter_context(tc.tile_pool(name="h", bufs=2))
    z_pool = ctx.enter_context(tc.tile_pool(name="z", bufs=2))
    zo_pool = ctx.enter_context(tc.tile_pool(name="zo", bufs=2))
    prior_pool = ctx.enter_context(tc.tile_pool(name="prior", bufs=4))
    small_pool = ctx.enter_context(tc.tile_pool(name="small", bufs=2))
    stat_pool = ctx.enter_context(tc.tile_pool(name="stat", bufs=2))
    b6_pool = ctx.enter_context(tc.tile_pool(name="b6", bufs=2))
    mm_ps = ctx.enter_context(tc.tile_pool(name="mm_ps", bufs=6,
                                           space="PSUM"))
    zt_ps = ctx.enter_context(tc.tile_pool(name="zt_ps", bufs=2,
                                           space="PSUM"))

    # --- constants ---
    ident_f = sb_const.tile([P, P], f32)
    masks.make_identity(nc, ident_f[:])
    ident = sb_const.tile([P, P], fp16)
    nc.vector.tensor_copy(ident[:], ident_f[:])
    NC = NU * MT      # 64 stat columns (4m+v)
    gtmp = sb_const.tile([P, MT], f32)
    btmp = sb_const.tile([P, MT], f32)
    nc.sync.dma_start(gtmp[:], Gd.rearrange("(m p) -> p m", p=P))
    nc.sync.dma_start(btmp[:], Bd.rearrange("(m p) -> p m", p=P))
    gx = sb_const.tile([P, NC], f32)
    bx = sb_const.tile([P, NC], f32)
    for v in range(NU):
        nc.vector.tensor_copy(gx[:, v:NC:NU], gtmp[:])
        nc.vector.tensor_copy(bx[:, v:NC:NU], btmp[:])
    half = sb_const.tile([P, NC], f32)
    nc.vector.memset(half[:], 0.5)

    # --- phase 0: weights (16 x 512KB fp16) ---
    WT = []
    for k in range(KT):
        w = wt_pool.tile([P, MT, P], fp16, tag=f"w{k}", name=f"w{k}")
        eng = nc.scalar if k % 2 == 0 else nc.sync
        eng.dma_start(w[:, :, :], W16[k])
        WT.append(w)

    if repeat > 1:
        rep_cm = tc.For_i(0, repeat, 1)
        rep_cm.__enter__()

    def emit_mm_group(s, mg, xt, B6, Hs, SC, SH, msum, dm, vr):
        """4 m-tiles: matmul (m-major, one psum tile at a time so banks
        rotate through the pool without group-sized stalls), bn_stats,
        stat math, fused apply."""
        pms = []
        for i in range(4):
            m = 4 * mg + i
            pm = mm_ps.tile([P, NU, P], f32, tag="mm", name=f"mm{s}_{mg}_{i}")
            pms.append(pm)
            for k in range(KT):
                nc.tensor.matmul(pm[:, :, :], WT[k][:, m, :],
                                 xt[:, k, :], start=(k == 0),
                                 stop=(k == KT - 1))
        if mode == 'mm':
            for i in range(4):
                nc.scalar.activation(B6[:, (4 * mg + i) * 24:
                                        (4 * mg + i) * 24 + 8],
                                     pms[i][:, 0, 0:8], AF.Identity)
            return
        # bn_stats per (m, v): 6 outputs = stats of two 64-row halves
        for i in range(4):
            m = 4 * mg + i
            for v in range(NU):
                base = 6 * (NU * m + v)
                nc.vector.bn_stats(B6[:, base:base + 6], pms[i][:, v, :])
        if mode == 'stats':
            return
        # stat math on this group's 16 (m,v) columns
        c0, c1 = 16 * mg, 16 * mg + 16
        b0, b1 = 96 * mg, 96 * mg + 96
        me_ap = B6[:, b0 + 1:b1:6]
        mo_ap = B6[:, b0 + 4:b1:6]
        m2e_ap = B6[:, b0 + 2:b1:6]
        m2o_ap = B6[:, b0 + 5:b1:6]
        msum_g = msum[:, c0:c1]
        dm_g = dm[:, c0:c1]
        vr_g = vr[:, c0:c1]
        sc_g = SC[:, c0:c1]
        sh_g = SH[:, c0:c1]
        nc.gpsimd.tensor_tensor(msum_g, me_ap, mo_ap, ALU.add)
        nc.gpsimd.tensor_tensor(dm_g, me_ap, mo_ap, ALU.subtract)
        nc.gpsimd.tensor_tensor(vr_g, m2e_ap, m2o_ap, ALU.add)
        nc.gpsimd.tensor_tensor(dm_g, dm_g, dm_g, ALU.mult)
        # var128 = M2e + M2o + 32*dm^2 ; vr = var128/128 + eps
        nc.vector.scalar_tensor_tensor(vr_g, dm_g, 32.0, vr_g,
                                       ALU.mult, ALU.add)
        nc.vector.tensor_scalar(vr_g, vr_g, 1.0 / P, EPS, ALU.mult, ALU.add)
        if RSQRT_MAGIC:
            # rstd = rsqrt(vr) via magic constant + 2 Newton steps
            yi = dm_g                                # reuse scratch
            nc.vector.tensor_scalar(yi.bitcast(i32), vr_g.bitcast(i32),
                                    1, None, ALU.arith_shift_right)
            nc.vector.tensor_scalar(yi.bitcast(i32), yi.bitcast(i32),
                                    -1, MAGIC, ALU.mult, ALU.add)
            vh_g = sc_g                              # scratch: v/2
            nc.gpsimd.tensor_tensor(vh_g, vr_g, half[:, c0:c1], ALU.mult)
            t_g = vr_g                               # scratch
            for _ in range(2):
                nc.gpsimd.tensor_tensor(t_g, yi, yi, ALU.mult)
                nc.gpsimd.tensor_tensor(t_g, t_g, vh_g, ALU.mult)
                nc.vector.tensor_scalar(t_g, t_g, -1.0, 1.5,
                                        ALU.mult, ALU.add)
                nc.gpsimd.tensor_tensor(yi, yi, t_g, ALU.mult)
        else:
            yi = vr_g
            nc.scalar.activation(vr_g, vr_g, AF.Sqrt)
            nc.vector.reciprocal(vr_g, vr_g)
        # SC = rstd*gamma ; SH = beta - (msum/2)*SC
        nc.gpsimd.tensor_tensor(sc_g, yi, gx[:, c0:c1], ALU.mult)
        nc.gpsimd.tensor_tensor(msum_g, msum_g, half[:, c0:c1], ALU.mult)
        nc.gpsimd.tensor_tensor(msum_g, msum_g, sc_g, ALU.mult)
        nc.gpsimd.tensor_tensor(sh_g, bx[:, c0:c1], msum_g, ALU.subtract)
        if mode == 'statmath':
            return
        # fused BN-apply + evacuate (ACT Identity): H fp16
        for i in range(4):
            m = 4 * mg + i
            for v in range(NU):
                col = NU * m + v
                nc.scalar.activation(Hs[m][:, v * P:(v + 1) * P],
                                     pms[i][:, v, :], AF.Identity,
                                     bias=SH[:, col:col + 1],
                                     scale=SC[:, col:col + 1])

    def emit_transposes(row0, Hs, zs, Cs, q):
        """PE-transpose quarter q for all u, prior-mult (DVE), top-8.
        Two u-quarters share one PSUM bank (2KB)."""
        for up in range(NU // 2):
            zt = zt_ps.tile([P, 2, 4 * P], fp16, tag="zt")
            for half in range(2):
                u = 2 * up + half
                for i in range(4):
                    m = 4 * q + i
                    nc.tensor.transpose(zt[:, half, i * P:(i + 1) * P],
                                        Hs[m][:, u * P:(u + 1) * P],
                                        ident[:])
                pch = prior_pool.tile([P, 4 * P], fp16, tag="prior")
                nc.sync.dma_start(
                    pch[:], PRI[row0 + u * P: row0 + (u + 1) * P,
                                q * 4 * P:(q + 1) * 4 * P])
                nc.vector.tensor_tensor(zs[u][:, q * 4 * P:(q + 1) * 4 * P],
                                        zt[:, half, :], pch[:], ALU.mult)
                nc.vector.max(Cs[u][:, 8 * q:8 * q + 8],
                              zs[u][:, 512 * q:512 * (q + 1)])

    def make_newton(zs, Cs, row0):
        its, relus, signs = [], [], []
        for u in range(NU):
            it = small_pool.tile([P, 8], f32, tag=f"it{u}",
                                 name=f"it{row0}_{u}")
            its.append(it)
            nc.vector.tensor_reduce(it[:, 0:1], Cs[u][:, 7:32:8], axis=AX.X,
                                    op=ALU.max, negate=True)     # tneg
            nc.vector.tensor_reduce(it[:, 5:6], Cs[u][:, 7:32:8], axis=AX.X,
                                    op=ALU.max)                  # tpos
            relus.append(small_pool.tile([P, 32], f32, tag=f"rl{u}",
                                         name=f"rl{row0}_{u}"))
            signs.append(small_pool.tile([P, 32], f32, tag=f"sg{u}",
                                         name=f"sg{row0}_{u}"))

        def step(u):
            it = its[u]
            tneg, racc, kacc = it[:, 0:1], it[:, 1:2], it[:, 2:3]
            krec, delta, tpos = it[:, 3:4], it[:, 4:5], it[:, 5:6]
            rr = it[:, 6:7]
            # accum_out reduces with op1, so op1 must be `add`:
            # racc = sum max(C, tpos) = sum relu(C - tpos) + 32*tpos
            nc.vector.tensor_scalar(relus[u][:], Cs[u][:], tpos, 0.0,
                                    ALU.max, ALU.add, accum_out=racc)
            nc.vector.tensor_scalar(signs[u][:], Cs[u][:], tpos, 0.0,
                                    ALU.is_gt, ALU.add, accum_out=kacc)
            nc.vector.reciprocal(krec, kacc)
            nc.vector.scalar_tensor_tensor(rr, tpos, -32.0, racc,
                                           ALU.mult, ALU.add)   # sum relu
            nc.vector.scalar_tensor_tensor(delta, rr, -1.0, krec,
                                           ALU.add, ALU.mult)
            nc.gpsimd.tensor_tensor(tneg, tneg, delta, ALU.subtract)
            nc.gpsimd.tensor_tensor(tpos, tpos, delta, ALU.add)

        def finish(u):
            zo = zo_pool.tile([P, D], f32, tag="zo", name=f"zo{row0}_{u}")
            nc.scalar.activation(zo[:], zs[u][:], AF.Relu,
                                 bias=its[u][:, 0:1])
            nc.scalar.dma_start(OUT[row0 + u * P: row0 + (u + 1) * P, :],
                                zo[:])
        return step, finish

    # --- pipeline over segments ---
    prev = None
    for s in range(NSEG):
        row0 = s * SEG
        xt = xt_pool.tile([P, KT, SEG], fp16, tag="xt", name=f"xt{s}")
        nc.sync.dma_start(xt[:, :, :], X16[s])

        B6 = b6_pool.tile([P, 6 * NC], f32, tag="B6", name=f"B6_{s}")
        SC = stat_pool.tile([P, NC], f32, tag="SC")
        SH = stat_pool.tile([P, NC], f32, tag="SH")
        msum = stat_pool.tile([P, NC], f32, tag="msum")
        dm = stat_pool.tile([P, NC], f32, tag="dm")
        vr = stat_pool.tile([P, NC], f32, tag="vr")
        Hs = [h_pool.tile([P, SEG], fp16, tag=f"h{m}", name=f"h{s}_{m}")
              for m in range(MT)]
        zs = [z_pool.tile([P, D], fp16, tag=f"z{u}", name=f"z{s}_{u}")
              for u in range(NU)]
        Cs = [small_pool.tile([P, 32], fp16, tag=f"C{u}", name=f"C{s}_{u}")
              for u in range(NU)]

        if prev is None or mode != 'full':
            for mg in range(4):
                emit_mm_group(s, mg, xt, B6, Hs, SC, SH, msum, dm, vr)
        else:
            prow0, pHs, pzs, pCs = prev
            for mg in range(4):
                emit_mm_group(s, mg, xt, B6, Hs, SC, SH, msum, dm, vr)
                emit_transposes(prow0, pHs, pzs, pCs, mg)
            step, finish = make_newton(pzs, pCs, prow0)
            for itn in range(NIT):
                for u in range(NU):
                    step(u)
            for u in range(NU):
                finish(u)
        prev = (row0, Hs, zs, Cs)

    if mode == 'full':
        prow0, pHs, pzs, pCs = prev
        for q in range(4):
            emit_transposes(prow0, pHs, pzs, pCs, q)
        step, finish = make_newton(pzs, pCs, prow0)
        for itn in range(NIT):
            for u in range(NU):
                step(u)
        for u in range(NU):
            finish(u)

    if repeat > 1:
        rep_cm.__exit__(None, None, None)


def build(repeat=1, mode='full'):
    nc = bacc.Bacc("TRN2", target_bir_lowering=False, debug=False)
    X16 = nc.dram_tensor("x16", [NSEG, P, KT, SEG], fp16,
                         kind="ExternalInput").ap()
    W16 = nc.dram_tensor("W16", [KT, P, MT, P], fp16,
                         kind="ExternalInput").ap()
    PRI = nc.dram_tensor("prior", [BL, D], fp16, kind="ExternalInput").ap()
    Gd = nc.dram_tensor("gamma", [D], f32, kind="ExternalInput").ap()
    Bd = nc.dram_tensor("beta", [D], f32, kind="ExternalInput").ap()
    OUT = nc.dram_tensor("out", [BL, D], f32, kind="ExternalOutput").ap()
    with tile.TileContext(nc) as tc, ExitStack() as ctx:
        _body(nc, tc, ctx, X16, W16, PRI, Gd, Bd, OUT, repeat=repeat,
              mode=mode)
    nc.compile()
    return nc


def prep_inputs(prior, x, W, gamma, beta):
    """Host-side packing shared by kernel() and test.py."""
    x = np.asarray(x, dtype=np.float32)
    W = np.asarray(W, dtype=np.float32)
    WT = np.ascontiguousarray(W.T).astype(np.float16)    # [i, o]
    W16 = np.ascontiguousarray(WT.reshape(KT, P, MT, P))
    prior16 = np.asarray(prior, dtype=np.float16)
    gamma = np.ascontiguousarray(gamma, dtype=np.float32)
    beta = np.ascontiguousarray(beta, dtype=np.float32)

    in_maps = []
    for c in range(N_CORES):
        sl = slice(c * BL, (c + 1) * BL)
        xT = x[sl].T.astype(np.float16)                  # [i, rows]
        # [s, p, k, b] with i = 128*k + p, rows = 512*s + b
        xp = xT.reshape(KT, P, NSEG, SEG).transpose(2, 1, 0, 3)
        in_maps.append({"x16": np.ascontiguousarray(xp),
                        "W16": W16,
                        "prior": np.ascontiguousarray(prior16[sl]),
                        "gamma": gamma, "beta": beta})
    return in_maps


_NC = None


def _run(inputs, trace=False, **kw):
    global _NC
    if _NC is None:
        _NC = build()
    in_maps = prep_inputs(inputs["prior"], inputs["x"], inputs["W"],
                          inputs["gamma"], inputs["beta"])
    res = run_bass_kernel_spmd(_NC, in_maps, list(range(N_CORES)),
                               trace=trace, **kw)
    out = np.concatenate([res.results[i]["out"] for i in range(N_CORES)],
                         axis=0)
    return out, res


def kernel(prior, x, W, gamma, beta):
    out, _ = _run({"prior": prior, "x": x, "W": W,
                   "gamma": gamma, "beta": beta})
    return out
